# revision 30
# baseline (speedup 1.0000x reference)
"""MoE block (E=8 experts, top-2) on 8 TRN2 NeuronCores — expert parallelism.

Strategy (per sharding hint): tokens are data-parallel across cores (1024
tokens/core); experts are sharded 1/core. Each core routes its own tokens in
f32 on device, packs them per destination expert via indirect-DMA scatter
(capacity 304 per (src,expert) pair), AllToAll-dispatches bf16 token rows,
runs its expert's FFN (gelu) in bf16 over everything it received, AllToAll
returns the expert outputs, and the owner combines the top-2 results with
router weights. The aux loss is reduced on device via a small AllGather.
"""

import numpy as np
import ml_dtypes

import concourse.bass as bass
import concourse.bacc as bacc
import concourse.tile as tile
from concourse import mybir
from concourse.bass import IndirectOffsetOnAxis

F32 = mybir.dt.float32
BF16 = mybir.dt.bfloat16
U32 = mybir.dt.uint32
AF = mybir.ActivationFunctionType
OP = mybir.AluOpType
AX = mybir.AxisListType

B, S, H, I, E, TOPK = 4, 2048, 1024, 4096, 8, 2
T = B * S                  # 8192 tokens
NCORES = 8
TLOC = T // NCORES         # 1024 tokens per core
P = 128
NTT = TLOC // P            # 8 token tiles per core
HT = H // P                # 8
IT = I // P                # 32
CAP = 304                  # per (src core, expert) dispatch capacity
RCV = NCORES * CAP         # 2432 rows received per expert
W1W = 160                  # wave-1 slots per pair
W2W = CAP - W1W            # 144
NW1 = NCORES * W1W         # 1280
NW2 = NCORES * W2W         # 1152
CHUNKS1 = [(0, 512), (512, 512), (1024, 256)]
CHUNKS2 = [(0, 512), (512, 512), (1024, 128)]
AUX_SCALE = float(E) / (T * TOPK * T)

_CACHED = {}


def _two_split(ap, which):
    """[P, 2n] interleaved view -> [P, n] strided column view (0 or 1)."""
    return (ap.rearrange("p (t two) -> p t two", two=2)[:, :, which:which + 1]
            .rearrange("p t o -> p (t o)"))


def _build_nc():
    nc = bacc.Bacc(num_devices=NCORES)

    # --- kernel I/O ---
    xb = nc.declare_dram_parameter("xb", [TLOC, H], BF16, isOutput=False)
    xt = nc.declare_dram_parameter("xt", [H, TLOC], F32, isOutput=False)
    wrt = nc.declare_dram_parameter("wrt", [P, HT * E], F32, isOutput=False)
    w1p = nc.declare_dram_parameter("w1p", [IT, P, H], BF16, isOutput=False)
    w2p = nc.declare_dram_parameter("w2p", [IT, P, H], BF16, isOutput=False)
    b1v = nc.declare_dram_parameter("b1v", [P, IT], F32, isOutput=False)
    b2bc = nc.declare_dram_parameter("b2bc", [P, H], F32, isOutput=False)
    zeroi = nc.declare_dram_parameter("zeroi", [RCV, H], BF16, isOutput=False)
    out = nc.declare_dram_parameter("out", [TLOC, H], F32, isOutput=True)
    auxo = nc.declare_dram_parameter("aux", [1, 1], F32, isOutput=True)

    # --- internal DRAM (collective bounce buffers) ---
    send = nc.dram_tensor("send", [RCV, H], BF16)
    send1 = nc.dram_tensor("send1", [NW1, H], BF16)
    send2 = nc.dram_tensor("send2", [NW2, H], BF16)
    recv1 = nc.dram_tensor("recv1", [NW1, H], BF16)
    recv2 = nc.dram_tensor("recv2", [NW2, H], BF16)
    ret = nc.dram_tensor("ret", [RCV, H], BF16)
    retr = nc.dram_tensor("retr", [RCV, H], BF16)
    stats_in = nc.dram_tensor("stats_in", [1, 2 * E], F32)
    totd = nc.dram_tensor("totd", [NTT, E], F32)
    card = nc.dram_tensor("card", [NTT, E], F32)
    stats_all = nc.dram_tensor("stats_all", [NCORES, 2 * E], F32,
                               addr_space="Shared")

    # --- constants (embedded in NEFF) ---
    lt = np.tril(np.ones((P, P), np.float32), -1).T.copy()  # lt[k,t]=1 iff k<t
    lt_d = nc.inline_tensor(lt, "lt_d")
    lt8 = np.tril(np.ones((NTT, NTT), np.float32), -1).T.copy()
    lt8_d = nc.inline_tensor(lt8, "lt8_d")
    ones_col_d = nc.inline_tensor(np.ones((P, 1), np.float32), "ones_col_d")
    ones_row_d = nc.inline_tensor(np.ones((1, P), np.float32), "ones_row_d")
    ones8_d = nc.inline_tensor(np.ones((NCORES, 1), np.float32), "ones8_d")
    id8_d = nc.inline_tensor(np.eye(E, dtype=np.float32), "id8_d")

    rg = [list(range(NCORES))]

    with tile.TileContext(nc) as tc:
        with tc.tile_pool(name="persist", bufs=1) as pers:
            # persistent tiles (router metadata reused by combine phase)
            wgt = pers.tile([P, 2 * NTT], F32)    # top-2 combine weights
            ridx = pers.tile([P, 2 * NTT], U32)   # top-2 dispatch rows
            b1_sb = pers.tile([P, IT], F32)
            w2_all = pers.tile([P, IT * H], BF16)  # resident W2^T (8.4 MB)

            # ---------------- router phase ----------------
            with (
                tc.tile_pool(name="rconst", bufs=1) as rc,
                tc.tile_pool(name="rxt", bufs=1) as rxt,
                tc.tile_pool(name="rwork", bufs=2) as rw,
                tc.tile_pool(name="plog", bufs=2, space="PSUM") as plog,
                tc.tile_pool(name="ppref", bufs=1, space="PSUM") as ppref,
                tc.tile_pool(name="ptot", bufs=1, space="PSUM") as ptot,
            ):
                # latency-critical loads first, on the SP HWDGE queue
                lt_sb = rc.tile([P, P], F32)
                nc.sync.dma_start(lt_sb[:], lt_d[:, :])
                ones_col = rc.tile([P, 1], F32)
                nc.sync.dma_start(ones_col[:], ones_col_d[:, :])
                ones_row = rc.tile([1, P], F32)
                nc.sync.dma_start(ones_row[:], ones_row_d[:, :])
                lt8_sb = rc.tile([NTT, NTT], F32)
                nc.sync.dma_start(lt8_sb[:], lt8_d[:, :])
                wrt_sb = rc.tile([P, HT * E], F32)
                nc.sync.dma_start(wrt_sb[:], wrt[:, :])
                id8_sb = rc.tile([E, E], F32)
                nc.sync.dma_start(id8_sb[:], id8_d[:, :])
                # xt split across both HWDGE rings to halve time-to-first-MM
                xt_sb = []
                for h in range(HT):
                    t = rxt.tile([P, TLOC], F32, tag=f"xt{h}")
                    eng = nc.sync if h < HT // 2 else nc.scalar
                    eng.dma_start(t[:], xt[h * P:(h + 1) * P, :])
                    xt_sb.append(t)
                xb_sb = []
                for tt in range(NTT):
                    t = rxt.tile([P, H], BF16, tag=f"xb{tt}")
                    nc.sync.dma_start(t[:], xb[tt * P:(tt + 1) * P, :])
                    xb_sb.append(t)

                # bulk loads on the ACT HWDGE queue (off the critical path)
                nc.scalar.dma_start(send[:, :], zeroi[:, :])
                nc.scalar.dma_start(b1_sb[:], b1v[:, :])
                nc.scalar.dma_start(
                    w2_all[:].rearrange("p (it h) -> p it h", it=IT),
                    w2p[:, :, :].rearrange("it p h -> p it h"))

                # router-wide tiles
                lg_all = rc.tile([P, NTT * E], F32)
                mxall = rc.tile([P, 2 * NTT], F32)
                ixall = rc.tile([P, 2 * NTT], U32)
                ef_all = rc.tile([P, 2 * NTT], F32)
                m1_all = rc.tile([P, NTT * E], F32)
                m2_all = rc.tile([P, NTT * E], F32)
                masks_all = rc.tile([P, NTT * E], F32)
                probs_all = rc.tile([P, NTT * E], F32)

                # R1a: logitsT via cheap-LDW matmuls, then transpose per tile
                lgT = rc.tile([E, TLOC], F32)
                for c in range(TLOC // 512):
                    plT = plog.tile([E, 512], F32, space="PSUM", tag="plT")
                    for h in range(HT):
                        nc.tensor.matmul(out=plT[:],
                                         lhsT=wrt_sb[:, h * E:(h + 1) * E],
                                         rhs=xt_sb[h][:, c * 512:(c + 1) * 512],
                                         start=(h == 0), stop=(h == HT - 1))
                    nc.vector.tensor_copy(lgT[:, c * 512:(c + 1) * 512], plT[:])
                for tt in range(NTT):
                    pl = plog.tile([P, E], F32, space="PSUM", tag="pl")
                    nc.tensor.transpose(out=pl[:],
                                        in_=lgT[:, tt * P:(tt + 1) * P],
                                        identity=id8_sb[:])
                    mx = rw.tile([P, E], F32, tag="mx")
                    nc.vector.max(out=mx[:], in_=pl[:])
                    ix = rw.tile([P, E], U32, tag="ix")
                    nc.vector.max_index(ix[:], mx[:], pl[:])
                    nc.vector.tensor_copy(lg_all[:, E * tt:E * (tt + 1)], pl[:])
                    nc.vector.tensor_copy(mxall[:, 2 * tt:2 * tt + 2],
                                          mx[:, 0:2])
                    nc.vector.tensor_copy(ixall[:, 2 * tt:2 * tt + 2],
                                          ix[:, 0:2])

                # R1b: batched across all 8 tiles ([P, 64] strided ops)
                lg3 = lg_all[:].rearrange("p (t e) -> p t e", e=E)
                v1r = (mxall[:].rearrange("p (t two) -> p t two", two=2)
                       [:, :, 0:1].to_broadcast([P, NTT, E]))
                v2r = (mxall[:].rearrange("p (t two) -> p t two", two=2)
                       [:, :, 1:2].to_broadcast([P, NTT, E]))
                m13 = m1_all[:].rearrange("p (t e) -> p t e", e=E)
                m23 = m2_all[:].rearrange("p (t e) -> p t e", e=E)
                nc.vector.tensor_tensor(out=m13, in0=lg3, in1=v1r,
                                        op=OP.is_equal)
                nc.vector.tensor_tensor(out=m23, in0=lg3, in1=v2r,
                                        op=OP.is_equal)
                nc.vector.tensor_add(masks_all[:], m1_all[:], m2_all[:])
                nc.vector.tensor_copy(ef_all[:], ixall[:])  # u32 -> f32

                sm_all = rc.tile([P, NTT * E], F32)
                sm3 = sm_all[:].rearrange("p (t e) -> p t e", e=E)
                nc.vector.tensor_tensor(out=sm3, in0=lg3, in1=v1r,
                                        op=OP.subtract)
                ex_all = rc.tile([P, NTT * E], F32)
                nc.scalar.activation(ex_all[:], sm_all[:], AF.Exp)
                z8 = rw.tile([P, NTT], F32, tag="z8")
                nc.vector.tensor_reduce(
                    out=z8[:], in_=ex_all[:].rearrange("p (t e) -> p t e", e=E),
                    axis=AX.X, op=OP.add)
                rz8 = rw.tile([P, NTT], F32, tag="rz8")
                nc.vector.reciprocal(rz8[:], z8[:])
                nc.vector.tensor_tensor(
                    out=probs_all[:].rearrange("p (t e) -> p t e", e=E),
                    in0=ex_all[:].rearrange("p (t e) -> p t e", e=E),
                    in1=rz8[:].rearrange("p (t o) -> p t o", o=1).to_broadcast(
                        [P, NTT, E]),
                    op=OP.mult)

                # top-2 weights: w1 = 1/(1+exp(v2-v1)); w2 = exp(v2-v1)*w1
                ddl = rw.tile([P, NTT], F32, tag="ddl")
                nc.vector.tensor_tensor(out=ddl[:], in0=_two_split(mxall[:], 1),
                                        in1=_two_split(mxall[:], 0),
                                        op=OP.subtract)
                ewl = rw.tile([P, NTT], F32, tag="ewl")
                nc.scalar.activation(ewl[:], ddl[:], AF.Exp)
                ssl = rw.tile([P, NTT], F32, tag="ssl")
                nc.vector.tensor_scalar_add(ssl[:], ewl[:], 1.0)
                nc.vector.reciprocal(_two_split(wgt[:], 0), ssl[:])
                nc.vector.tensor_tensor(out=_two_split(wgt[:], 1), in0=ewl[:],
                                        in1=_two_split(wgt[:], 0), op=OP.mult)

                # R2: per-tile totals -> cross-tile carries + global stats
                ptt = ptot.tile([1, NTT * E], F32, space="PSUM", tag="ptt")
                nc.tensor.matmul(out=ptt[:], lhsT=ones_col[:], rhs=masks_all[:],
                                 start=True, stop=True)
                tot_sb = rw.tile([1, NTT * E], F32, tag="tot")
                nc.vector.tensor_copy(tot_sb[:], ptt[:])
                nc.sync.dma_start(
                    totd[:, :].rearrange("n e -> (n e)"), tot_sb[:, :])
                tot8 = rw.tile([NTT, E], F32, tag="tot8")
                nc.sync.dma_start(tot8[:], totd[:, :])
                pcar = ptot.tile([NTT, E], F32, space="PSUM", tag="pcar")
                nc.tensor.matmul(out=pcar[:], lhsT=lt8_sb[:], rhs=tot8[:],
                                 start=True, stop=True)
                carry_sb = rw.tile([NTT, E], F32, tag="carry")
                nc.vector.tensor_copy(carry_sb[:], pcar[:])
                nc.sync.dma_start(card[:, :], carry_sb[:])
                carr1 = rw.tile([1, NTT * E], F32, tag="carr1")
                nc.sync.dma_start(carr1[:],
                                  card[:, :].rearrange("n e -> (n e)"))

                ptp = ptot.tile([1, NTT * E], F32, space="PSUM", tag="ptp")
                nc.tensor.matmul(out=ptp[:], lhsT=ones_col[:], rhs=probs_all[:],
                                 start=True, stop=True)
                st_sb = rw.tile([1, 2 * E], F32, tag="stsb")
                nc.vector.tensor_reduce(
                    out=st_sb[:, 0:E],
                    in_=tot_sb[:].rearrange("o (n e) -> o e n", n=NTT),
                    axis=AX.X, op=OP.add)
                nc.vector.tensor_reduce(
                    out=st_sb[:, E:2 * E],
                    in_=ptp[:].rearrange("o (n e) -> o e n", n=NTT),
                    axis=AX.X, op=OP.add)
                nc.sync.dma_start(stats_in[:, :], st_sb[:])

                # R3: slot prefix sums (one batched matmul pair) + row ids
                pfall = ppref.tile([P, NTT * E], F32, space="PSUM", tag="pfall")
                nc.tensor.matmul(out=pfall[:], lhsT=lt_sb[:], rhs=masks_all[:],
                                 start=True, stop=False)
                nc.tensor.matmul(out=pfall[:], lhsT=ones_row[:], rhs=carr1[:],
                                 start=False, stop=True)
                sc1 = rc.tile([P, NTT * E], F32)
                nc.vector.tensor_mul(sc1[:], pfall[:], m1_all[:])
                s1a = rw.tile([P, NTT], F32, tag="s1a")
                nc.vector.tensor_reduce(
                    out=s1a[:], in_=sc1[:].rearrange("p (t e) -> p t e", e=E),
                    axis=AX.X, op=OP.add)
                sc2 = rc.tile([P, NTT * E], F32)
                nc.vector.tensor_mul(sc2[:], pfall[:], m2_all[:])
                s2a = rw.tile([P, NTT], F32, tag="s2a")
                nc.vector.tensor_reduce(
                    out=s2a[:], in_=sc2[:].rearrange("p (t e) -> p t e", e=E),
                    axis=AX.X, op=OP.add)
                nc.vector.tensor_scalar_min(s1a[:], s1a[:], float(CAP - 1))
                nc.vector.tensor_scalar_min(s2a[:], s2a[:], float(CAP - 1))
                r_all = rc.tile([P, 2 * NTT], F32)
                t1a = rw.tile([P, NTT], F32, tag="t1a")
                nc.vector.tensor_scalar_mul(t1a[:], _two_split(ef_all[:], 0),
                                            float(CAP))
                nc.vector.tensor_add(_two_split(r_all[:], 0), t1a[:], s1a[:])
                t2a = rw.tile([P, NTT], F32, tag="t2a")
                nc.vector.tensor_scalar_mul(t2a[:], _two_split(ef_all[:], 1),
                                            float(CAP))
                nc.vector.tensor_add(_two_split(r_all[:], 1), t2a[:], s2a[:])
                nc.vector.tensor_copy(ridx[:], r_all[:])  # f32 -> u32

                for tt in range(NTT):
                    nc.gpsimd.indirect_dma_start(
                        out=send[:, :],
                        out_offset=IndirectOffsetOnAxis(
                            ap=ridx[:, 2 * tt:2 * tt + 1], axis=0),
                        in_=xb_sb[tt][:, :], in_offset=None)
                    nc.gpsimd.indirect_dma_start(
                        out=send[:, :],
                        out_offset=IndirectOffsetOnAxis(
                            ap=ridx[:, 2 * tt + 1:2 * tt + 2], axis=0),
                        in_=xb_sb[tt][:, :], in_offset=None)

            # repack the wave halves into contiguous per-wave buffers
            sv = send[:, :].rearrange("(c s) h -> c s h", s=CAP)
            nc.sync.dma_start(
                send1[:, :].rearrange("(c s) h -> c s h", s=W1W),
                sv[:, 0:W1W, :])
            nc.scalar.dma_start(
                send2[:, :].rearrange("(c s) h -> c s h", s=W2W),
                sv[:, W1W:CAP, :])
            tc.strict_bb_all_engine_barrier()
            nc.gpsimd.collective_compute(
                "AllToAll", OP.bypass, replica_groups=rg,
                ins=[send1[:, :]], outs=[recv1[:, :]])

            # ---------------- expert FFN phase ----------------
            with (
                tc.tile_pool(name="xrt", bufs=2 * HT) as xrtp,
                tc.tile_pool(name="w1s", bufs=3) as w1s,
                tc.tile_pool(name="hid", bufs=IT + 4) as hidp,
                tc.tile_pool(name="yout", bufs=3) as yp,
                tc.tile_pool(name="psA", bufs=2, space="PSUM") as psA,
                tc.tile_pool(name="psB", bufs=2, space="PSUM") as psB,
            ):
                def ffn_chunks(chunk_list, rcv, wbase, wsz):
                  for r0, tcs in chunk_list:
                    xrt = []
                    for h in range(HT):
                        t = xrtp.tile([P, tcs], BF16, tag="xrt")
                        nc.sync.dma_start_transpose(
                            t[:], rcv[r0:r0 + tcs, h * P:(h + 1) * P])
                        xrt.append(t)
                    hid = []
                    for it in range(IT):
                        w1t = w1s.tile([P, H], BF16, tag="w1t")
                        nc.sync.dma_start(w1t[:], w1p[it, :, :])
                        pf1 = psA.tile([P, tcs], F32, space="PSUM", tag="pf1")
                        for h in range(HT):
                            nc.tensor.matmul(out=pf1[:],
                                             lhsT=w1t[:, h * P:(h + 1) * P],
                                             rhs=xrt[h][:],
                                             start=(h == 0), stop=(h == HT - 1))
                        ht_ = hidp.tile([P, tcs], BF16, tag="hid")
                        nc.scalar.activation(ht_[:], pf1[:], AF.Gelu,
                                             bias=b1_sb[:, it:it + 1], scale=1.0)
                        hid.append(ht_)
                    for m in range(tcs // P):
                        ysb = yp.tile([P, H], BF16, tag="ysb")
                        pn0 = psB.tile([P, 512], F32, space="PSUM", tag="pn0")
                        pn1 = psB.tile([P, 512], F32, space="PSUM", tag="pn1")
                        for it in range(IT):
                            lhs = hid[it][:, m * P:(m + 1) * P]
                            nc.tensor.matmul(
                                out=pn0[:], lhsT=lhs,
                                rhs=w2_all[:, it * H:it * H + 512],
                                start=(it == 0), stop=(it == IT - 1),
                                skip_group_check=True)
                            nc.tensor.matmul(
                                out=pn1[:], lhsT=lhs,
                                rhs=w2_all[:, it * H + 512:(it + 1) * H],
                                start=(it == 0), stop=(it == IT - 1),
                                skip_group_check=True)
                        nc.vector.tensor_copy(ysb[:, 0:512], pn0[:])
                        nc.vector.tensor_copy(ysb[:, 512:1024], pn1[:])
                        a = r0 + m * P
                        rem, cur = P, a
                        while rem:
                            c, off = cur // wsz, cur % wsz
                            n = min(rem, wsz - off)
                            nc.sync.dma_start(
                                ret[c * CAP + wbase + off:
                                    c * CAP + wbase + off + n, :],
                                ysb[cur - a:cur - a + n, :])
                            rem -= n
                            cur += n

                ffn_chunks(CHUNKS1, recv1, 0, W1W)
                # wave-2 tokens arrive while wave-1 chunks compute
                nc.gpsimd.collective_compute(
                    "AllToAll", OP.bypass, replica_groups=rg,
                    ins=[send2[:, :]], outs=[recv2[:, :]])
                ffn_chunks(CHUNKS2, recv2, W1W, W2W)

            # stats AllGather after the FFN: its xbar-vs-collective
            # serialization must never gate the FFN's transpose DMAs
            nc.gpsimd.collective_compute(
                "AllGather", OP.bypass, replica_groups=rg,
                ins=[stats_in[:, :]], outs=[stats_all[:, :]])

            tc.strict_bb_all_engine_barrier()
            nc.gpsimd.collective_compute(
                "AllToAll", OP.bypass, replica_groups=rg,
                ins=[ret[:, :]], outs=[retr[:, :]])

            # aux loss: E * sum(counts/(T*K) * probsum/T) over global stats
            with (
                tc.tile_pool(name="auxw", bufs=1) as axw,
                tc.tile_pool(name="paux", bufs=1, space="PSUM") as paux,
            ):
                sa = axw.tile([NCORES, 2 * E], F32)
                nc.sync.dma_start(sa[:], stats_all[:, :])
                ones8 = axw.tile([NCORES, 1], F32)
                nc.sync.dma_start(ones8[:], ones8_d[:, :])
                pav = paux.tile([1, 2 * E], F32, space="PSUM")
                nc.tensor.matmul(out=pav[:], lhsT=ones8[:], rhs=sa[:],
                                 start=True, stop=True)
                pav_sb = axw.tile([1, 2 * E], F32)
                nc.vector.tensor_copy(pav_sb[:], pav[:])
                avj = axw.tile([1, E], F32)
                axs = axw.tile([1, 1], F32)
                nc.vector.tensor_mul(avj[:], pav_sb[:, 0:E], pav_sb[:, E:2 * E])
                nc.vector.tensor_reduce(out=axs[:], in_=avj[:],
                                        axis=AX.X, op=OP.add)
                nc.vector.tensor_scalar_mul(axs[:], axs[:], AUX_SCALE)
                nc.sync.dma_start(auxo[:, :], axs[:])

                # ---------------- combine phase ----------------
                with (
                    tc.tile_pool(name="cmb", bufs=3) as cb,
                    tc.tile_pool(name="cconst", bufs=1) as cc,
                ):
                    b2_sb = cc.tile([P, H], F32)
                    nc.sync.dma_start(b2_sb[:], b2bc[:, :])
                    for tt in range(NTT):
                        ya = cb.tile([P, H], BF16, tag="ya")
                        nc.gpsimd.indirect_dma_start(
                            out=ya[:, :], out_offset=None, in_=retr[:, :],
                            in_offset=IndirectOffsetOnAxis(
                                ap=ridx[:, 2 * tt:2 * tt + 1], axis=0))
                        yb = cb.tile([P, H], BF16, tag="yb")
                        nc.gpsimd.indirect_dma_start(
                            out=yb[:, :], out_offset=None, in_=retr[:, :],
                            in_offset=IndirectOffsetOnAxis(
                                ap=ridx[:, 2 * tt + 1:2 * tt + 2], axis=0))
                        tg = cb.tile([P, H], F32, tag="tg")
                        nc.vector.scalar_tensor_tensor(
                            out=tg[:], in0=ya[:],
                            scalar=wgt[:, 2 * tt:2 * tt + 1], in1=b2_sb[:],
                            op0=OP.mult, op1=OP.add)
                        og = cb.tile([P, H], F32, tag="og")
                        nc.vector.scalar_tensor_tensor(
                            out=og[:], in0=yb[:],
                            scalar=wgt[:, 2 * tt + 1:2 * tt + 2], in1=tg[:],
                            op0=OP.mult, op1=OP.add)
                        nc.sync.dma_start(out[tt * P:(tt + 1) * P, :], og[:])

    return nc


def get_nc():
    if "nc" not in _CACHED:
        nc = _build_nc()
        nc.finalize()   # Bacc.compile(): wait legalization, reg alloc, ...
        _CACHED["nc"] = nc
    return _CACHED["nc"]


def make_in_maps(x, Wr, W1, b1, W2, b2):
    """Shard the full inputs into the 8 per-core input dicts."""
    x = np.ascontiguousarray(x, np.float32)
    flat = x.reshape(T, H)
    # wrt packed: wrtp[p, h*E+e] = Wr[e, h*128+p]
    wrtp = np.ascontiguousarray(
        Wr.T.astype(np.float32).reshape(HT, P, E).transpose(1, 0, 2)
        .reshape(P, HT * E))
    wrt = np.ascontiguousarray(Wr.T, np.float32)  # [H, E]
    in_maps = []
    for c in range(NCORES):
        sh = flat[c * TLOC:(c + 1) * TLOC]
        w1t = W1[c].T.astype(ml_dtypes.bfloat16)    # [H, I]
        # w1p[it, p, h*128+j] = W1T[h*128+p, it*128+j]
        w1pk = np.ascontiguousarray(
            w1t.reshape(HT, P, IT, P).transpose(2, 1, 0, 3).reshape(IT, P, H))
        w2pk = np.ascontiguousarray(
            W2[c].T.astype(ml_dtypes.bfloat16).reshape(IT, P, H))
        in_maps.append({
            "xb": sh.astype(ml_dtypes.bfloat16),
            "xt": np.ascontiguousarray(sh.T),
            "wrt": wrtp,
            "w1p": w1pk,
            "w2p": w2pk,
            "b1v": np.ascontiguousarray(
                b1[c].astype(np.float32).reshape(IT, P).T),
            "b2bc": np.broadcast_to(
                b2[c].astype(np.float32), (P, H)).copy(),
            "zeroi": np.zeros((RCV, H), ml_dtypes.bfloat16),
        })
    return in_maps


def assemble(results):
    outs = [np.asarray(r["out"], np.float32) for r in results]
    full = np.concatenate(outs, axis=0).reshape(B, S, H)
    aux = np.float32(np.asarray(results[0]["aux"]).reshape(-1)[0])
    return full, aux


def kernel(x, Wr, W1, b1, W2, b2):
    from concourse.bass_utils import run_bass_kernel_spmd
    nc = get_nc()
    in_maps = make_in_maps(x, Wr, W1, b1, W2, b2)
    res = run_bass_kernel_spmd(nc, in_maps, list(range(NCORES)))
    return assemble(res.results)


# revision 32
# speedup vs baseline: 1.0562x; 1.0562x over previous
"""MoE block (E=8 experts, top-2) on 8 TRN2 NeuronCores — expert parallelism.

Strategy (per sharding hint): tokens are data-parallel across cores (1024
tokens/core); experts are sharded 1/core. Each core routes its own tokens in
f32 on device, packs them per destination expert via indirect-DMA scatter
(capacity 304 per (src,expert) pair), AllToAll-dispatches bf16 token rows,
runs its expert's FFN (gelu) in bf16 over everything it received, AllToAll
returns the expert outputs, and the owner combines the top-2 results with
router weights. The aux loss is reduced on device via a small AllGather.
"""

import numpy as np
import ml_dtypes

import concourse.bass as bass
import concourse.bacc as bacc
import concourse.tile as tile
from concourse import mybir
from concourse.bass import IndirectOffsetOnAxis

F32 = mybir.dt.float32
BF16 = mybir.dt.bfloat16
U32 = mybir.dt.uint32
AF = mybir.ActivationFunctionType
OP = mybir.AluOpType
AX = mybir.AxisListType

B, S, H, I, E, TOPK = 4, 2048, 1024, 4096, 8, 2
T = B * S                  # 8192 tokens
NCORES = 8
TLOC = T // NCORES         # 1024 tokens per core
P = 128
NTT = TLOC // P            # 8 token tiles per core
HT = H // P                # 8
IT = I // P                # 32
CAP = 304                  # per (src core, expert) dispatch capacity
RCV = NCORES * CAP         # 2432 rows received per expert
CHUNKS = [(0, 512), (512, 512), (1024, 512), (1536, 512), (2048, 384)]
AUX_SCALE = float(E) / (T * TOPK * T)

_CACHED = {}


def _two_split(ap, which):
    """[P, 2n] interleaved view -> [P, n] strided column view (0 or 1)."""
    return (ap.rearrange("p (t two) -> p t two", two=2)[:, :, which:which + 1]
            .rearrange("p t o -> p (t o)"))


def _build_nc():
    nc = bacc.Bacc(num_devices=NCORES)

    # --- kernel I/O ---
    xb = nc.declare_dram_parameter("xb", [TLOC, H], BF16, isOutput=False)
    xt = nc.declare_dram_parameter("xt", [H, TLOC], F32, isOutput=False)
    wrt = nc.declare_dram_parameter("wrt", [P, HT * E], F32, isOutput=False)
    w1p = nc.declare_dram_parameter("w1p", [IT, P, H], BF16, isOutput=False)
    w2p = nc.declare_dram_parameter("w2p", [IT, P, H], BF16, isOutput=False)
    b1v = nc.declare_dram_parameter("b1v", [P, IT], F32, isOutput=False)
    b2bc = nc.declare_dram_parameter("b2bc", [P, H], F32, isOutput=False)
    zeroi = nc.declare_dram_parameter("zeroi", [RCV, H], BF16, isOutput=False)
    out = nc.declare_dram_parameter("out", [TLOC, H], F32, isOutput=True)
    auxo = nc.declare_dram_parameter("aux", [1, 1], F32, isOutput=True)

    # --- internal DRAM (collective bounce buffers) ---
    send = nc.dram_tensor("send", [RCV, H], BF16)
    recv = nc.dram_tensor("recv", [RCV, H], BF16)
    ret = nc.dram_tensor("ret", [RCV, H], BF16)
    retr = nc.dram_tensor("retr", [RCV, H], BF16)
    stats_in = nc.dram_tensor("stats_in", [1, 2 * E], F32)
    totd = nc.dram_tensor("totd", [NTT, E], F32)
    card = nc.dram_tensor("card", [NTT, E], F32)
    stats_all = nc.dram_tensor("stats_all", [NCORES, 2 * E], F32,
                               addr_space="Shared")

    # --- constants (embedded in NEFF) ---
    lt = np.tril(np.ones((P, P), np.float32), -1).T.copy()  # lt[k,t]=1 iff k<t
    lt_d = nc.inline_tensor(lt, "lt_d")
    lt8 = np.tril(np.ones((NTT, NTT), np.float32), -1).T.copy()
    lt8_d = nc.inline_tensor(lt8, "lt8_d")
    ones_col_d = nc.inline_tensor(np.ones((P, 1), np.float32), "ones_col_d")
    ones_row_d = nc.inline_tensor(np.ones((1, P), np.float32), "ones_row_d")
    ones8_d = nc.inline_tensor(np.ones((NCORES, 1), np.float32), "ones8_d")
    id8_d = nc.inline_tensor(np.eye(E, dtype=np.float32), "id8_d")

    rg = [list(range(NCORES))]

    with tile.TileContext(nc) as tc:
        with tc.tile_pool(name="persist", bufs=1) as pers:
            # persistent tiles (router metadata reused by combine phase)
            wgt = pers.tile([P, 2 * NTT], F32)    # top-2 combine weights
            ridx = pers.tile([P, 2 * NTT], U32)   # top-2 dispatch rows
            b1_sb = pers.tile([P, IT], F32)
            w2_all = pers.tile([P, IT * H], BF16)  # resident W2^T (8.4 MB)

            # ---------------- router phase ----------------
            with (
                tc.tile_pool(name="rconst", bufs=1) as rc,
                tc.tile_pool(name="rxt", bufs=1) as rxt,
                tc.tile_pool(name="rwork", bufs=2) as rw,
                tc.tile_pool(name="plog", bufs=2, space="PSUM") as plog,
                tc.tile_pool(name="ppref", bufs=1, space="PSUM") as ppref,
                tc.tile_pool(name="ptot", bufs=1, space="PSUM") as ptot,
            ):
                # latency-critical loads first, on the SP HWDGE queue
                lt_sb = rc.tile([P, P], F32)
                nc.sync.dma_start(lt_sb[:], lt_d[:, :])
                ones_col = rc.tile([P, 1], F32)
                nc.sync.dma_start(ones_col[:], ones_col_d[:, :])
                ones_row = rc.tile([1, P], F32)
                nc.sync.dma_start(ones_row[:], ones_row_d[:, :])
                lt8_sb = rc.tile([NTT, NTT], F32)
                nc.sync.dma_start(lt8_sb[:], lt8_d[:, :])
                wrt_sb = rc.tile([P, HT * E], F32)
                nc.sync.dma_start(wrt_sb[:], wrt[:, :])
                id8_sb = rc.tile([E, E], F32)
                nc.sync.dma_start(id8_sb[:], id8_d[:, :])
                # xt split across both HWDGE rings to halve time-to-first-MM
                xt_sb = []
                for h in range(HT):
                    t = rxt.tile([P, TLOC], F32, tag=f"xt{h}")
                    eng = nc.sync if h < HT // 2 else nc.scalar
                    eng.dma_start(t[:], xt[h * P:(h + 1) * P, :])
                    xt_sb.append(t)
                xb_sb = []
                for tt in range(NTT):
                    t = rxt.tile([P, H], BF16, tag=f"xb{tt}")
                    nc.sync.dma_start(t[:], xb[tt * P:(tt + 1) * P, :])
                    xb_sb.append(t)

                # bulk loads on the ACT HWDGE queue (off the critical path)
                nc.scalar.dma_start(send[:, :], zeroi[:, :])
                nc.scalar.dma_start(b1_sb[:], b1v[:, :])
                nc.scalar.dma_start(
                    w2_all[:].rearrange("p (it h) -> p it h", it=IT),
                    w2p[:, :, :].rearrange("it p h -> p it h"))

                # router-wide tiles
                lg_all = rc.tile([P, NTT * E], F32)
                mxall = rc.tile([P, 2 * NTT], F32)
                ixall = rc.tile([P, 2 * NTT], U32)
                ef_all = rc.tile([P, 2 * NTT], F32)
                m1_all = rc.tile([P, NTT * E], F32)
                m2_all = rc.tile([P, NTT * E], F32)
                masks_all = rc.tile([P, NTT * E], F32)
                probs_all = rc.tile([P, NTT * E], F32)

                # R1a: logitsT via cheap-LDW matmuls, then transpose per tile
                lgT = rc.tile([E, TLOC], F32)
                for c in range(TLOC // 512):
                    plT = plog.tile([E, 512], F32, space="PSUM", tag="plT")
                    for h in range(HT):
                        nc.tensor.matmul(out=plT[:],
                                         lhsT=wrt_sb[:, h * E:(h + 1) * E],
                                         rhs=xt_sb[h][:, c * 512:(c + 1) * 512],
                                         start=(h == 0), stop=(h == HT - 1))
                    nc.vector.tensor_copy(lgT[:, c * 512:(c + 1) * 512], plT[:])
                for tt in range(NTT):
                    pl = plog.tile([P, E], F32, space="PSUM", tag="pl")
                    nc.tensor.transpose(out=pl[:],
                                        in_=lgT[:, tt * P:(tt + 1) * P],
                                        identity=id8_sb[:])
                    mx = rw.tile([P, E], F32, tag="mx")
                    nc.vector.max(out=mx[:], in_=pl[:])
                    ix = rw.tile([P, E], U32, tag="ix")
                    nc.vector.max_index(ix[:], mx[:], pl[:])
                    nc.vector.tensor_copy(lg_all[:, E * tt:E * (tt + 1)], pl[:])
                    nc.vector.tensor_copy(mxall[:, 2 * tt:2 * tt + 2],
                                          mx[:, 0:2])
                    nc.vector.tensor_copy(ixall[:, 2 * tt:2 * tt + 2],
                                          ix[:, 0:2])

                # R1b: batched across all 8 tiles ([P, 64] strided ops)
                lg3 = lg_all[:].rearrange("p (t e) -> p t e", e=E)
                v1r = (mxall[:].rearrange("p (t two) -> p t two", two=2)
                       [:, :, 0:1].to_broadcast([P, NTT, E]))
                v2r = (mxall[:].rearrange("p (t two) -> p t two", two=2)
                       [:, :, 1:2].to_broadcast([P, NTT, E]))
                m13 = m1_all[:].rearrange("p (t e) -> p t e", e=E)
                m23 = m2_all[:].rearrange("p (t e) -> p t e", e=E)
                nc.vector.tensor_tensor(out=m13, in0=lg3, in1=v1r,
                                        op=OP.is_equal)
                nc.vector.tensor_tensor(out=m23, in0=lg3, in1=v2r,
                                        op=OP.is_equal)
                nc.vector.tensor_add(masks_all[:], m1_all[:], m2_all[:])
                nc.vector.tensor_copy(ef_all[:], ixall[:])  # u32 -> f32

                sm_all = rc.tile([P, NTT * E], F32)
                sm3 = sm_all[:].rearrange("p (t e) -> p t e", e=E)
                nc.vector.tensor_tensor(out=sm3, in0=lg3, in1=v1r,
                                        op=OP.subtract)
                ex_all = rc.tile([P, NTT * E], F32)
                nc.scalar.activation(ex_all[:], sm_all[:], AF.Exp)
                z8 = rw.tile([P, NTT], F32, tag="z8")
                nc.vector.tensor_reduce(
                    out=z8[:], in_=ex_all[:].rearrange("p (t e) -> p t e", e=E),
                    axis=AX.X, op=OP.add)
                rz8 = rw.tile([P, NTT], F32, tag="rz8")
                nc.vector.reciprocal(rz8[:], z8[:])
                nc.vector.tensor_tensor(
                    out=probs_all[:].rearrange("p (t e) -> p t e", e=E),
                    in0=ex_all[:].rearrange("p (t e) -> p t e", e=E),
                    in1=rz8[:].rearrange("p (t o) -> p t o", o=1).to_broadcast(
                        [P, NTT, E]),
                    op=OP.mult)

                # top-2 weights: w1 = 1/(1+exp(v2-v1)); w2 = exp(v2-v1)*w1
                ddl = rw.tile([P, NTT], F32, tag="ddl")
                nc.vector.tensor_tensor(out=ddl[:], in0=_two_split(mxall[:], 1),
                                        in1=_two_split(mxall[:], 0),
                                        op=OP.subtract)
                ewl = rw.tile([P, NTT], F32, tag="ewl")
                nc.scalar.activation(ewl[:], ddl[:], AF.Exp)
                ssl = rw.tile([P, NTT], F32, tag="ssl")
                nc.vector.tensor_scalar_add(ssl[:], ewl[:], 1.0)
                nc.vector.reciprocal(_two_split(wgt[:], 0), ssl[:])
                nc.vector.tensor_tensor(out=_two_split(wgt[:], 1), in0=ewl[:],
                                        in1=_two_split(wgt[:], 0), op=OP.mult)

                # R2: per-tile totals -> cross-tile carries + global stats
                ptt = ptot.tile([1, NTT * E], F32, space="PSUM", tag="ptt")
                nc.tensor.matmul(out=ptt[:], lhsT=ones_col[:], rhs=masks_all[:],
                                 start=True, stop=True)
                tot_sb = rw.tile([1, NTT * E], F32, tag="tot")
                nc.vector.tensor_copy(tot_sb[:], ptt[:])
                nc.sync.dma_start(
                    totd[:, :].rearrange("n e -> (n e)"), tot_sb[:, :])
                tot8 = rw.tile([NTT, E], F32, tag="tot8")
                nc.sync.dma_start(tot8[:], totd[:, :])
                pcar = ptot.tile([NTT, E], F32, space="PSUM", tag="pcar")
                nc.tensor.matmul(out=pcar[:], lhsT=lt8_sb[:], rhs=tot8[:],
                                 start=True, stop=True)
                carry_sb = rw.tile([NTT, E], F32, tag="carry")
                nc.vector.tensor_copy(carry_sb[:], pcar[:])
                nc.sync.dma_start(card[:, :], carry_sb[:])
                carr1 = rw.tile([1, NTT * E], F32, tag="carr1")
                nc.sync.dma_start(carr1[:],
                                  card[:, :].rearrange("n e -> (n e)"))

                ptp = ptot.tile([1, NTT * E], F32, space="PSUM", tag="ptp")
                nc.tensor.matmul(out=ptp[:], lhsT=ones_col[:], rhs=probs_all[:],
                                 start=True, stop=True)
                st_sb = rw.tile([1, 2 * E], F32, tag="stsb")
                nc.vector.tensor_reduce(
                    out=st_sb[:, 0:E],
                    in_=tot_sb[:].rearrange("o (n e) -> o e n", n=NTT),
                    axis=AX.X, op=OP.add)
                nc.vector.tensor_reduce(
                    out=st_sb[:, E:2 * E],
                    in_=ptp[:].rearrange("o (n e) -> o e n", n=NTT),
                    axis=AX.X, op=OP.add)
                nc.sync.dma_start(stats_in[:, :], st_sb[:])

                # R3: slot prefix sums (one batched matmul pair) + row ids
                pfall = ppref.tile([P, NTT * E], F32, space="PSUM", tag="pfall")
                nc.tensor.matmul(out=pfall[:], lhsT=lt_sb[:], rhs=masks_all[:],
                                 start=True, stop=False)
                nc.tensor.matmul(out=pfall[:], lhsT=ones_row[:], rhs=carr1[:],
                                 start=False, stop=True)
                sc1 = rc.tile([P, NTT * E], F32)
                nc.vector.tensor_mul(sc1[:], pfall[:], m1_all[:])
                s1a = rw.tile([P, NTT], F32, tag="s1a")
                nc.vector.tensor_reduce(
                    out=s1a[:], in_=sc1[:].rearrange("p (t e) -> p t e", e=E),
                    axis=AX.X, op=OP.add)
                sc2 = rc.tile([P, NTT * E], F32)
                nc.vector.tensor_mul(sc2[:], pfall[:], m2_all[:])
                s2a = rw.tile([P, NTT], F32, tag="s2a")
                nc.vector.tensor_reduce(
                    out=s2a[:], in_=sc2[:].rearrange("p (t e) -> p t e", e=E),
                    axis=AX.X, op=OP.add)
                nc.vector.tensor_scalar_min(s1a[:], s1a[:], float(CAP - 1))
                nc.vector.tensor_scalar_min(s2a[:], s2a[:], float(CAP - 1))
                r_all = rc.tile([P, 2 * NTT], F32)
                t1a = rw.tile([P, NTT], F32, tag="t1a")
                nc.vector.tensor_scalar_mul(t1a[:], _two_split(ef_all[:], 0),
                                            float(CAP))
                nc.vector.tensor_add(_two_split(r_all[:], 0), t1a[:], s1a[:])
                t2a = rw.tile([P, NTT], F32, tag="t2a")
                nc.vector.tensor_scalar_mul(t2a[:], _two_split(ef_all[:], 1),
                                            float(CAP))
                nc.vector.tensor_add(_two_split(r_all[:], 1), t2a[:], s2a[:])
                nc.vector.tensor_copy(ridx[:], r_all[:])  # f32 -> u32

                for tt in range(NTT):
                    nc.gpsimd.indirect_dma_start(
                        out=send[:, :],
                        out_offset=IndirectOffsetOnAxis(
                            ap=ridx[:, 2 * tt:2 * tt + 1], axis=0),
                        in_=xb_sb[tt][:, :], in_offset=None)
                    nc.gpsimd.indirect_dma_start(
                        out=send[:, :],
                        out_offset=IndirectOffsetOnAxis(
                            ap=ridx[:, 2 * tt + 1:2 * tt + 2], axis=0),
                        in_=xb_sb[tt][:, :], in_offset=None)

            nc.gpsimd.collective_compute(
                "AllToAll", OP.bypass, replica_groups=rg,
                ins=[send[:, :]], outs=[recv[:, :]])

            # ---------------- expert FFN phase ----------------
            with (
                tc.tile_pool(name="xrt", bufs=2 * HT) as xrtp,
                tc.tile_pool(name="w1s", bufs=3) as w1s,
                tc.tile_pool(name="hid", bufs=IT + 4) as hidp,
                tc.tile_pool(name="yout", bufs=3) as yp,
                tc.tile_pool(name="psA", bufs=2, space="PSUM") as psA,
                tc.tile_pool(name="psB", bufs=2, space="PSUM") as psB,
            ):
                for r0, tcs in CHUNKS:
                    xrt = []
                    for h in range(HT):
                        t = xrtp.tile([P, tcs], BF16, tag="xrt")
                        nc.sync.dma_start_transpose(
                            t[:], recv[r0:r0 + tcs, h * P:(h + 1) * P])
                        xrt.append(t)
                    hid = []
                    for it in range(IT):
                        w1t = w1s.tile([P, H], BF16, tag="w1t")
                        nc.sync.dma_start(w1t[:], w1p[it, :, :])
                        pf1 = psA.tile([P, tcs], F32, space="PSUM", tag="pf1")
                        for h in range(HT):
                            nc.tensor.matmul(out=pf1[:],
                                             lhsT=w1t[:, h * P:(h + 1) * P],
                                             rhs=xrt[h][:],
                                             start=(h == 0), stop=(h == HT - 1))
                        ht_ = hidp.tile([P, tcs], BF16, tag="hid")
                        nc.scalar.activation(ht_[:], pf1[:], AF.Gelu,
                                             bias=b1_sb[:, it:it + 1], scale=1.0)
                        hid.append(ht_)
                    for m in range(tcs // P):
                        ysb = yp.tile([P, H], BF16, tag="ysb")
                        pn0 = psB.tile([P, 512], F32, space="PSUM", tag="pn0")
                        pn1 = psB.tile([P, 512], F32, space="PSUM", tag="pn1")
                        for it in range(IT):
                            lhs = hid[it][:, m * P:(m + 1) * P]
                            nc.tensor.matmul(
                                out=pn0[:], lhsT=lhs,
                                rhs=w2_all[:, it * H:it * H + 512],
                                start=(it == 0), stop=(it == IT - 1),
                                skip_group_check=True)
                            nc.tensor.matmul(
                                out=pn1[:], lhsT=lhs,
                                rhs=w2_all[:, it * H + 512:(it + 1) * H],
                                start=(it == 0), stop=(it == IT - 1),
                                skip_group_check=True)
                        nc.vector.tensor_copy(ysb[:, 0:512], pn0[:])
                        nc.vector.tensor_copy(ysb[:, 512:1024], pn1[:])
                        nc.sync.dma_start(ret[r0 + m * P:r0 + (m + 1) * P, :],
                                          ysb[:])

            # stats AllGather after the FFN: its xbar-vs-collective
            # serialization must never gate the FFN's transpose DMAs
            nc.gpsimd.collective_compute(
                "AllGather", OP.bypass, replica_groups=rg,
                ins=[stats_in[:, :]], outs=[stats_all[:, :]])

            nc.gpsimd.collective_compute(
                "AllToAll", OP.bypass, replica_groups=rg,
                ins=[ret[:, :]], outs=[retr[:, :]])

            # aux loss: E * sum(counts/(T*K) * probsum/T) over global stats
            with (
                tc.tile_pool(name="auxw", bufs=1) as axw,
                tc.tile_pool(name="paux", bufs=1, space="PSUM") as paux,
            ):
                sa = axw.tile([NCORES, 2 * E], F32)
                nc.sync.dma_start(sa[:], stats_all[:, :])
                ones8 = axw.tile([NCORES, 1], F32)
                nc.sync.dma_start(ones8[:], ones8_d[:, :])
                pav = paux.tile([1, 2 * E], F32, space="PSUM")
                nc.tensor.matmul(out=pav[:], lhsT=ones8[:], rhs=sa[:],
                                 start=True, stop=True)
                pav_sb = axw.tile([1, 2 * E], F32)
                nc.vector.tensor_copy(pav_sb[:], pav[:])
                avj = axw.tile([1, E], F32)
                axs = axw.tile([1, 1], F32)
                nc.vector.tensor_mul(avj[:], pav_sb[:, 0:E], pav_sb[:, E:2 * E])
                nc.vector.tensor_reduce(out=axs[:], in_=avj[:],
                                        axis=AX.X, op=OP.add)
                nc.vector.tensor_scalar_mul(axs[:], axs[:], AUX_SCALE)
                nc.sync.dma_start(auxo[:, :], axs[:])

                # ---------------- combine phase ----------------
                with (
                    tc.tile_pool(name="cmb", bufs=3) as cb,
                    tc.tile_pool(name="cconst", bufs=1) as cc,
                ):
                    b2_sb = cc.tile([P, H], F32)
                    nc.sync.dma_start(b2_sb[:], b2bc[:, :])
                    for tt in range(NTT):
                        ya = cb.tile([P, H], BF16, tag="ya")
                        nc.gpsimd.indirect_dma_start(
                            out=ya[:, :], out_offset=None, in_=retr[:, :],
                            in_offset=IndirectOffsetOnAxis(
                                ap=ridx[:, 2 * tt:2 * tt + 1], axis=0))
                        yb = cb.tile([P, H], BF16, tag="yb")
                        nc.gpsimd.indirect_dma_start(
                            out=yb[:, :], out_offset=None, in_=retr[:, :],
                            in_offset=IndirectOffsetOnAxis(
                                ap=ridx[:, 2 * tt + 1:2 * tt + 2], axis=0))
                        tg = cb.tile([P, H], F32, tag="tg")
                        nc.vector.scalar_tensor_tensor(
                            out=tg[:], in0=ya[:],
                            scalar=wgt[:, 2 * tt:2 * tt + 1], in1=b2_sb[:],
                            op0=OP.mult, op1=OP.add)
                        og = cb.tile([P, H], F32, tag="og")
                        nc.vector.scalar_tensor_tensor(
                            out=og[:], in0=yb[:],
                            scalar=wgt[:, 2 * tt + 1:2 * tt + 2], in1=tg[:],
                            op0=OP.mult, op1=OP.add)
                        nc.sync.dma_start(out[tt * P:(tt + 1) * P, :], og[:])

    return nc


def get_nc():
    if "nc" not in _CACHED:
        nc = _build_nc()
        nc.finalize()   # Bacc.compile(): wait legalization, reg alloc, ...
        _CACHED["nc"] = nc
    return _CACHED["nc"]


def make_in_maps(x, Wr, W1, b1, W2, b2):
    """Shard the full inputs into the 8 per-core input dicts."""
    x = np.ascontiguousarray(x, np.float32)
    flat = x.reshape(T, H)
    # wrt packed: wrtp[p, h*E+e] = Wr[e, h*128+p]
    wrtp = np.ascontiguousarray(
        Wr.T.astype(np.float32).reshape(HT, P, E).transpose(1, 0, 2)
        .reshape(P, HT * E))
    wrt = np.ascontiguousarray(Wr.T, np.float32)  # [H, E]
    in_maps = []
    for c in range(NCORES):
        sh = flat[c * TLOC:(c + 1) * TLOC]
        w1t = W1[c].T.astype(ml_dtypes.bfloat16)    # [H, I]
        # w1p[it, p, h*128+j] = W1T[h*128+p, it*128+j]
        w1pk = np.ascontiguousarray(
            w1t.reshape(HT, P, IT, P).transpose(2, 1, 0, 3).reshape(IT, P, H))
        w2pk = np.ascontiguousarray(
            W2[c].T.astype(ml_dtypes.bfloat16).reshape(IT, P, H))
        in_maps.append({
            "xb": sh.astype(ml_dtypes.bfloat16),
            "xt": np.ascontiguousarray(sh.T),
            "wrt": wrtp,
            "w1p": w1pk,
            "w2p": w2pk,
            "b1v": np.ascontiguousarray(
                b1[c].astype(np.float32).reshape(IT, P).T),
            "b2bc": np.broadcast_to(
                b2[c].astype(np.float32), (P, H)).copy(),
            "zeroi": np.zeros((RCV, H), ml_dtypes.bfloat16),
        })
    return in_maps


def assemble(results):
    outs = [np.asarray(r["out"], np.float32) for r in results]
    full = np.concatenate(outs, axis=0).reshape(B, S, H)
    aux = np.float32(np.asarray(results[0]["aux"]).reshape(-1)[0])
    return full, aux


def kernel(x, Wr, W1, b1, W2, b2):
    from concourse.bass_utils import run_bass_kernel_spmd
    nc = get_nc()
    in_maps = make_in_maps(x, Wr, W1, b1, W2, b2)
    res = run_bass_kernel_spmd(nc, in_maps, list(range(NCORES)))
    return assemble(res.results)
